# revision 12
# baseline (speedup 1.0000x reference)
"""Trainium2 Bass kernel for nn_FFTConv:
  out[b,p,h,w] = sum_c z[b,c,h,w]*filt[c,p,h,w] + sum_c bias[c,p,h,w]
with complex64 z[8,32,128,128], filt/bias[32,32,128,128].

Strategy
--------
Shard the spatial H dim across the 8 NeuronCores (16 rows each) -- pure data
parallelism, zero replication, no collectives.

Each output pixel needs a tiny complex matmul out[p,b] = F(px)^T @ z(px) with
K=c=32, M=p=32, N=b=8, plus a bias-channel sum. We pack these on the PE's
32x32 sub-array grid:

  pixel slot (a in {0,1}, j in {0..3}) -> PE rows 64a:64a+64, cols 32j:32j+32
  lhsT_A = [filt.re(32 rows); bias.re(32 rows)]  (64 x 32p)
  rhs_A  = [z.re | z.im ; ones | zeros]          (64 x 16)   cols = (u2, b8)
  lhsT_B = [filt.im; bias.im], rhs_B = [-z.im | z.re ; zeros | ones]
  psum[32j:32j+32, ...16] = lhsT_A.T@rhs_A + lhsT_B.T@rhs_B
    -> re = sum fr*zr - fi*zi + sum br ; im = sum fr*zi + fi*zr + sum bi

The extra 32 "ones/zeros" rhs rows fold the bias-channel reduction into the
same matmul for free. 8 pixels run concurrently across the sub-array grid.

Host-side numpy only reorders data (transpose/interleave) into pixel-major
DMA-friendly layouts; all arithmetic happens on device.

Layouts (per core, f32):
  local pixel px = h_local*128 + w in [0,2048); group = px//8; k=px%8;
  a=k//4; j=k%4; super-batch sb = group//32 (8 sbs x 32 groups x 8 px).
  wa/wb [sb,128,4096]: part = 64a + r (r<32: filt c=r; r>=32: bias c=r-32);
                       free = g*128 + j*32 + p
  zd [sb,2,32,4096]:   part rows 64a+c; free = g*128+j*32+blk*16+u*8+b
  out_dev [sb,128,1024]: part = 32j+p; free = g*32 + a*16 + u*8 + b
"""

import numpy as np

B, C, P, H, W = 8, 32, 32, 128, 128
NCORES = 8
HPER = H // NCORES          # 16
PX = HPER * W               # 2048
NSB = 8                     # super-batches per core
NG = 32                     # groups per super-batch

_DT = np.float32


def _pixelize(arr_t, n):
    """arr_t: [h, w, ...]; returns [sb, g, a, j, ...] for core n's h-strip."""
    a = arr_t[HPER * n:HPER * (n + 1)]
    a = a.reshape(PX, *a.shape[2:])
    return a.reshape(NSB, NG, 2, 4, *a.shape[1:])


def _prepare_inputs(z, filt, bias):
    zr = np.ascontiguousarray(z.real).astype(_DT)
    zi = np.ascontiguousarray(z.imag).astype(_DT)
    fr = np.ascontiguousarray(filt.real).astype(_DT)
    fi = np.ascontiguousarray(filt.imag).astype(_DT)
    br = np.ascontiguousarray(bias.real).astype(_DT)
    bi = np.ascontiguousarray(bias.imag).astype(_DT)

    fr_t = fr.transpose(2, 3, 0, 1)   # [h, w, c, p]
    fi_t = fi.transpose(2, 3, 0, 1)
    br_t = br.transpose(2, 3, 0, 1)
    bi_t = bi.transpose(2, 3, 0, 1)
    zr_t = zr.transpose(2, 3, 1, 0)   # [h, w, c, b]
    zi_t = zi.transpose(2, 3, 1, 0)

    in_maps = []
    for n in range(NCORES):
        # [sb, g, a, j, c, x] -> [sb, a, c, g, j, x]
        perm = (0, 2, 4, 1, 3, 5)
        frp = _pixelize(fr_t, n).transpose(perm)
        fip = _pixelize(fi_t, n).transpose(perm)
        brp = _pixelize(br_t, n).transpose(perm)
        bip = _pixelize(bi_t, n).transpose(perm)
        wa = np.concatenate([frp, brp], axis=2).reshape(NSB, 128, 4096)
        wb = np.concatenate([fip, bip], axis=2).reshape(NSB, 128, 4096)

        zrp = _pixelize(zr_t, n).transpose(perm)
        zip_ = _pixelize(zi_t, n).transpose(perm)
        zd = np.empty((NSB, 2, 32, NG, 4, 2, 2, 8), _DT)
        zd[..., 0, 0, :] = zrp
        zd[..., 0, 1, :] = zip_
        zd[..., 1, 0, :] = -zip_
        zd[..., 1, 1, :] = zrp
        zd = zd.reshape(NSB, 2, 32, 4096)

        in_maps.append({
            "wa": np.ascontiguousarray(wa),
            "wb": np.ascontiguousarray(wb),
            "zd": np.ascontiguousarray(zd),
        })
    return in_maps


def _assemble_output(res_list):
    out = np.empty((B, P, H, W), np.complex64)
    for n in range(NCORES):
        # psum partition q = 64a + 32jj + p ; free f = g*32 + k*16 + u*8 + b
        arr = res_list[n].reshape(NSB, 2, 2, 32, NG, 2, 2, 8)
        # [sb, a, jj, p, g, k, u, b] -> [b, p, sb, g, a, jj, k, u]
        arr = arr.transpose(7, 3, 0, 4, 1, 2, 5, 6)
        arr = np.ascontiguousarray(arr).reshape(B, P, HPER, W, 2)
        out[:, :, HPER * n:HPER * (n + 1), :] = arr[..., 0] + 1j * arr[..., 1]
    return out


_NC_CACHE = [None]


def _build_bass():
    if _NC_CACHE[0] is not None:
        return _NC_CACHE[0]
    import concourse.mybir as mybir
    import concourse.tile as tile
    from concourse import bacc

    f32 = mybir.dt.float32
    nc = bacc.Bacc("TRN2", target_bir_lowering=False, debug=False)
    wa_d = nc.dram_tensor("wa", [NSB, 128, 4096], f32, kind="ExternalInput")
    wb_d = nc.dram_tensor("wb", [NSB, 128, 4096], f32, kind="ExternalInput")
    zd_d = nc.dram_tensor("zd", [NSB, 2, 32, 4096], f32, kind="ExternalInput")
    out_d = nc.dram_tensor("out_dev", [NSB, 128, 1024], f32,
                           kind="ExternalOutput")

    # Persistent double-buffered z tensors (fixed SBUF allocations, outside
    # the Tile pools so their slots can't be recycled).
    zb_handles = [
        nc.alloc_sbuf_tensor(f"zb{i}", [128, 4096], f32) for i in range(2)
    ]

    with tile.TileContext(nc) as tc:
        with (
            tc.tile_pool(name="wp", bufs=3) as wpool,
            tc.tile_pool(name="op", bufs=3) as opool,
            tc.tile_pool(name="pp", bufs=8, space="PSUM") as pspool,
        ):
            # const rows written once per z buffer
            zbufs = []
            for i in range(2):
                zb = zb_handles[i][:]
                for a in range(2):
                    cr = zb[64 * a + 32: 64 * a + 64]          # [32, 4096]
                    crq = cr.rearrange("r (m q) -> r m q", q=32)
                    nc.vector.memset(cr, 0.0)
                    nc.vector.memset(crq[:, :, 0:8], 1.0)      # blk0/u0
                    nc.vector.memset(crq[:, :, 24:32], 1.0)    # blk1/u1
                zbufs.append(zb)

            for sb in range(NSB):
                wa_t = wpool.tile([128, 4096], f32, name="wa_t", tag="wa_t")
                wb_t = wpool.tile([128, 4096], f32, name="wb_t", tag="wb_t")
                nc.sync.dma_start(out=wa_t, in_=wa_d[sb])
                nc.sync.dma_start(out=wb_t, in_=wb_d[sb])
                zb = zbufs[sb % 2]
                for a in range(2):
                    nc.sync.dma_start(out=zb[64 * a:64 * a + 32, :],
                                      in_=zd_d[sb, a])

                o_t = opool.tile([128, 1024], f32, name="o_t", tag="o_t")
                for gg in range(2):                 # 16 groups per psum tile
                    # Full-bank psum tile (2048B/partition) so accumulation
                    # zero-regions align exactly with 32-partition col-group
                    # slices. Col-groups are statically partitioned between
                    # row-halves (a=0 -> cols {0,32}, a=1 -> cols {64,96}):
                    # concurrent fp32 matmuls from different row-groups into
                    # the same col-group crash the PE (fp32 LO/HI two-pass),
                    # so each col-group is fed by exactly one row-half.
                    ps = pspool.tile([128, 512], f32, name="ps", tag="ps")
                    for g16 in range(16):
                        g = gg * 16 + g16
                        base = g * 128
                        # pixel (a, jj, k): a=row-half, jj=col slot, k=serial
                        # pair idx; j := px%4 = jj*2+k keeps the W/z layouts
                        # unchanged. Per k: the 4 (a,jj) slots' A-matmuls
                        # open 4 groups on 4 distinct col-groups, then the 4
                        # B-matmuls close them.
                        for k in range(2):
                            for half, w_t in ((0, wa_t), (1, wb_t)):
                                for a in range(2):
                                    for jj in range(2):
                                        j = jj * 2 + k
                                        fo = base + j * 32
                                        cg = 64 * a + 32 * jj
                                        lhs = w_t[64 * a:64 * a + 64,
                                                  fo:fo + 32]
                                        rhs = zb[64 * a:64 * a + 64,
                                                 fo + 16 * half:
                                                 fo + 16 * half + 16]
                                        po = ps[cg:cg + 32,
                                                g16 * 32 + k * 16:
                                                g16 * 32 + k * 16 + 16]
                                        nc.tensor.matmul(
                                            po, lhs, rhs,
                                            start=(half == 0),
                                            stop=(half == 1),
                                            tile_position=(64 * a, cg),
                                            # sim's global group-check shadow
                                            # mis-addresses partition-sliced
                                            # psum outputs; the per-tensor
                                            # pending-zero numerics are still
                                            # modeled faithfully
                                            skip_group_check=True,
                                        )
                    if gg % 2 == 0:
                        nc.vector.tensor_copy(
                            o_t[:, gg * 512:(gg + 1) * 512], ps)
                    else:
                        nc.scalar.copy(
                            o_t[:, gg * 512:(gg + 1) * 512], ps)
                nc.sync.dma_start(out=out_d[sb], in_=o_t)

    nc.compile()
    _NC_CACHE[0] = nc
    return nc


def run(z, filt, bias, trace=False, trace_kwargs=None):
    """Returns (out, BassKernelResults)."""
    from concourse.bass_utils import run_bass_kernel_spmd
    in_maps = _prepare_inputs(z, filt, bias)
    nc = _build_bass()
    bkr = run_bass_kernel_spmd(
        nc, in_maps, core_ids=list(range(NCORES)),
        trace=trace, **(trace_kwargs or {}),
    )
    out = _assemble_output([r["out_dev"] for r in bkr.results])
    return out, bkr


def kernel(z, filt, bias):
    out, _ = run(np.asarray(z), np.asarray(filt), np.asarray(bias))
    return out


# revision 13
# speedup vs baseline: 1.3893x; 1.3893x over previous
"""Trainium2 Bass kernel for nn_FFTConv:
  out[b,p,h,w] = sum_c z[b,c,h,w]*filt[c,p,h,w] + sum_c bias[c,p,h,w]
with complex64 z[8,32,128,128], filt/bias[32,32,128,128].

Strategy
--------
Shard the spatial H dim across the 8 NeuronCores (16 rows each) -- pure data
parallelism, zero replication, no collectives.

Each output pixel needs a tiny complex matmul out[p,b] = F(px)^T @ z(px) with
K=c=32, M=p=32, N=b=8, plus a bias-channel sum. We fold the whole complex
product AND the bias reduction into ONE K=128 real matmul per pixel:

  lhsT (128 x 32p) rows = [filt.re(32c); bias.re(32c); filt.im(32c); bias.im(32c)]
  rhs  (128 x 16)  cols = (u2, b8):
      rows  0-31  : [ z.re | z.im ]
      rows 32-63  : [  1   |  0   ]   (bias.re only contributes to re)
      rows 64-95  : [-z.im | z.re ]
      rows 96-127 : [  0   |  1   ]
  out[p, (u,b)]  ->  re = fr*zr - fi*zi + sum(br) ; im = fr*zi + fi*zr + sum(bi)

Pixels are packed 4-at-a-time across the PE's four 32-col sub-array groups
(tile_position=(0, 32j)); the constant ones/zeros rows live in persistent
SBUF buffers written once (memset), so they cost no DMA.

NOTE: concurrent fp32 matmuls from *different* row-groups into the same
col-group crash the PE (fp32 LO/HI two-pass) -- with K=128 every matmul uses
all rows as a single instruction, which is safe; col-groups are concurrent.

Host-side numpy only reorders data (transpose/interleave/negate) into
pixel-major DMA-friendly layouts; all arithmetic happens on device.

Layouts (per core, f32):
  local pixel px = h_local*128 + w in [0,2048); group g = px//8;
  within group: j = px%4 (col slot), k = (px%8)//4 (serial);
  super-batch sb = g//32 (8 sbs x 32 groups x 8 px).
  wc [sb,128,8192]: row = 32*blk + c (blk: fr,br,fi,bi); free = g*256+k*128+j*32+p
  zd [sb,2,32,4096]: half 0 -> SBUF rows 0-31 (u0=zr,u1=zi),
                     half 1 -> rows 64-95 (u0=-zi,u1=zr);
                     free = g*128 + k*64 + j*16 + u*8 + b
  out_dev [sb,128,1024]: part = 32j+p; free = g*32 + k*16 + u*8 + b
"""

import numpy as np

B, C, P, H, W = 8, 32, 32, 128, 128
NCORES = 8
HPER = H // NCORES          # 16
PX = HPER * W               # 2048
NSB = 8                     # super-batches per core
NG = 32                     # groups per super-batch

_DT = np.float32


def _pixelize(arr_t, n):
    """arr_t: [h, w, ...]; returns [sb, g, k, j, ...] for core n's h-strip."""
    a = arr_t[HPER * n:HPER * (n + 1)]
    a = a.reshape(PX, *a.shape[2:])
    return a.reshape(NSB, NG, 2, 4, *a.shape[1:])


def _prepare_inputs(z, filt, bias):
    zr = np.ascontiguousarray(z.real).astype(_DT)
    zi = np.ascontiguousarray(z.imag).astype(_DT)
    fr = np.ascontiguousarray(filt.real).astype(_DT)
    fi = np.ascontiguousarray(filt.imag).astype(_DT)
    br = np.ascontiguousarray(bias.real).astype(_DT)
    bi = np.ascontiguousarray(bias.imag).astype(_DT)

    fr_t = fr.transpose(2, 3, 0, 1)   # [h, w, c, p]
    fi_t = fi.transpose(2, 3, 0, 1)
    br_t = br.transpose(2, 3, 0, 1)
    bi_t = bi.transpose(2, 3, 0, 1)
    zr_t = zr.transpose(2, 3, 1, 0)   # [h, w, c, b]
    zi_t = zi.transpose(2, 3, 1, 0)

    in_maps = []
    for n in range(NCORES):
        # [sb, g, k, j, c, x] -> [sb, c, g, k, j, x]
        perm = (0, 4, 1, 2, 3, 5)
        frp = _pixelize(fr_t, n).transpose(perm)
        fip = _pixelize(fi_t, n).transpose(perm)
        brp = _pixelize(br_t, n).transpose(perm)
        bip = _pixelize(bi_t, n).transpose(perm)
        # rows = (blk4, c32): [fr; br; fi; bi]
        wc = np.stack([frp, brp, fip, bip], axis=1)    # [sb, blk, c, g, k, j, p]
        wc = wc.reshape(NSB, 128, 8192)

        zrp = _pixelize(zr_t, n).transpose(perm)       # [sb, c, g, k, j, b]
        zip_ = _pixelize(zi_t, n).transpose(perm)
        zd = np.empty((NSB, 2, 32, NG, 2, 4, 2, 8), _DT)
        zd[:, 0, ..., 0, :] = zrp
        zd[:, 0, ..., 1, :] = zip_
        zd[:, 1, ..., 0, :] = -zip_
        zd[:, 1, ..., 1, :] = zrp
        zd = zd.reshape(NSB, 2, 32, 4096)

        in_maps.append({
            "wc": np.ascontiguousarray(wc),
            "zd": np.ascontiguousarray(zd),
        })
    return in_maps


def _assemble_output(res_list):
    out = np.empty((B, P, H, W), np.complex64)
    for n in range(NCORES):
        # part q = 32j + p ; free f = g*32 + k*16 + u*8 + b
        arr = res_list[n].reshape(NSB, 4, 32, NG, 2, 2, 8)
        # [sb, j, p, g, k, u, b] -> [b, p, sb, g, k, j, u]
        arr = arr.transpose(6, 2, 0, 3, 4, 1, 5)
        arr = np.ascontiguousarray(arr).reshape(B, P, HPER, W, 2)
        out[:, :, HPER * n:HPER * (n + 1), :] = arr[..., 0] + 1j * arr[..., 1]
    return out


_NC_CACHE = [None]


def _build_bass():
    if _NC_CACHE[0] is not None:
        return _NC_CACHE[0]
    import concourse.mybir as mybir
    import concourse.tile as tile
    from concourse import bacc

    f32 = mybir.dt.float32
    nc = bacc.Bacc("TRN2", target_bir_lowering=False, debug=False)
    wc_d = nc.dram_tensor("wc", [NSB, 128, 8192], f32, kind="ExternalInput")
    zd_d = nc.dram_tensor("zd", [NSB, 2, 32, 4096], f32, kind="ExternalInput")
    out_d = nc.dram_tensor("out_dev", [NSB, 128, 1024], f32,
                           kind="ExternalOutput")

    # Persistent double-buffered z tensors (fixed SBUF allocations, outside
    # the Tile pools so their slots can't be recycled).
    zb_handles = [
        nc.alloc_sbuf_tensor(f"zb{i}", [128, 4096], f32) for i in range(2)
    ]

    with tile.TileContext(nc) as tc:
        with (
            tc.tile_pool(name="wp", bufs=2) as wpool,
            tc.tile_pool(name="op", bufs=3) as opool,
            tc.tile_pool(name="pp", bufs=8, space="PSUM") as pspool,
        ):
            # const rhs rows, written once per z buffer:
            # rows 32-63 : per 16-block [1]*8 + [0]*8   (bias.re -> re col)
            # rows 96-127: per 16-block [0]*8 + [1]*8   (bias.im -> im col)
            zbufs = []
            for i in range(2):
                zb = zb_handles[i][:]
                for rbase, off in ((32, 0), (96, 8)):
                    cr = zb[rbase:rbase + 32]                  # [32, 4096]
                    crq = cr.rearrange("r (m q) -> r m q", q=16)
                    nc.vector.memset(cr, 0.0)
                    nc.vector.memset(crq[:, :, off:off + 8], 1.0)
                zbufs.append(zb)

            for sb in range(NSB):
                wc_t = wpool.tile([128, 8192], f32, name="wc_t", tag="wc_t")
                nc.sync.dma_start(out=wc_t, in_=wc_d[sb])
                zb = zbufs[sb % 2]
                for hf in range(2):
                    nc.sync.dma_start(out=zb[64 * hf:64 * hf + 32, :],
                                      in_=zd_d[sb, hf])

                o_t = opool.tile([128, 1024], f32, name="o_t", tag="o_t")
                for gg in range(2):                 # 16 groups per psum tile
                    # Full-bank psum tile (2048B/partition) so accumulation
                    # zero-regions align exactly with 32-partition col-group
                    # slices.
                    ps = pspool.tile([128, 512], f32, name="ps", tag="ps")
                    for g16 in range(16):
                        g = gg * 16 + g16
                        # one K=128 matmul per pixel; consecutive matmuls hit
                        # different col-groups and run concurrently
                        for k in range(2):
                            for j in range(4):
                                fo = g * 256 + k * 128 + j * 32
                                zo = g * 128 + k * 64 + j * 16
                                lhs = wc_t[:, fo:fo + 32]
                                rhs = zb[:, zo:zo + 16]
                                po = ps[32 * j:32 * j + 32,
                                        g16 * 32 + k * 16:
                                        g16 * 32 + k * 16 + 16]
                                nc.tensor.matmul(
                                    po, lhs, rhs, start=True, stop=True,
                                    tile_position=(0, 32 * j),
                                    # sim's global group-check shadow
                                    # mis-addresses partition-sliced psum
                                    # outputs; the per-tensor pending-zero
                                    # numerics are still modeled faithfully
                                    skip_group_check=True,
                                )
                    if gg % 2 == 0:
                        nc.vector.tensor_copy(
                            o_t[:, gg * 512:(gg + 1) * 512], ps)
                    else:
                        nc.scalar.copy(
                            o_t[:, gg * 512:(gg + 1) * 512], ps)
                nc.sync.dma_start(out=out_d[sb], in_=o_t)

    nc.compile()
    _NC_CACHE[0] = nc
    return nc


def run(z, filt, bias, trace=False, trace_kwargs=None):
    """Returns (out, BassKernelResults)."""
    from concourse.bass_utils import run_bass_kernel_spmd
    in_maps = _prepare_inputs(z, filt, bias)
    nc = _build_bass()
    bkr = run_bass_kernel_spmd(
        nc, in_maps, core_ids=list(range(NCORES)),
        trace=trace, **(trace_kwargs or {}),
    )
    out = _assemble_output([r["out_dev"] for r in bkr.results])
    return out, bkr


def kernel(z, filt, bias):
    out, _ = run(np.asarray(z), np.asarray(filt), np.asarray(bias))
    return out


# revision 14
# speedup vs baseline: 1.7265x; 1.2427x over previous
"""Trainium2 Bass kernel for nn_FFTConv:
  out[b,p,h,w] = sum_c z[b,c,h,w]*filt[c,p,h,w] + sum_c bias[c,p,h,w]
with complex64 z[8,32,128,128], filt/bias[32,32,128,128].

Strategy
--------
Shard the spatial H dim across the 8 NeuronCores (16 rows each) -- pure data
parallelism, zero replication, no collectives.

Each output pixel needs a tiny complex matmul out[p,b] = F(px)^T @ z(px) with
K=c=32, M=p=32, N=b=8, plus a bias-channel sum. We fold the whole complex
product AND the bias reduction into ONE K=128 real matmul per pixel:

  lhsT (128 x 32p) rows = [filt.re(32c); bias.re(32c); filt.im(32c); bias.im(32c)]
  rhs  (128 x 16)  cols = (u2, b8):
      rows  0-31  : [ z.re | z.im ]
      rows 32-63  : [  1   |  0   ]   (bias.re only contributes to re)
      rows 64-95  : [-z.im | z.re ]
      rows 96-127 : [  0   |  1   ]
  out[p, (u,b)]  ->  re = fr*zr - fi*zi + sum(br) ; im = fr*zi + fi*zr + sum(bi)

Pixels are packed 4-at-a-time across the PE's four 32-col sub-array groups
(tile_position=(0, 32j)); the constant ones/zeros rows live in persistent
SBUF buffers written once (memset), so they cost no DMA.

NOTE: concurrent fp32 matmuls from *different* row-groups into the same
col-group crash the PE (fp32 LO/HI two-pass) -- with K=128 every matmul uses
all rows as a single instruction, which is safe; col-groups are concurrent.

Host-side numpy only reorders data (transpose/interleave/negate) into
pixel-major DMA-friendly layouts; all arithmetic happens on device.

Layouts (per core, f32):
  local pixel px = h_local*128 + w in [0,2048); group g = px//8;
  within group: j = px%4 (col slot), k = (px%8)//4 (serial);
  super-batch sb = g//32 (8 sbs x 32 groups x 8 px).
  wc [sb,128,8192]: row = 32*blk + c (blk: fr,br,fi,bi); free = g*256+k*128+j*32+p
  zd [sb,2,32,4096]: half 0 -> SBUF rows 0-31 (u0=zr,u1=zi),
                     half 1 -> rows 64-95 (u0=-zi,u1=zr);
                     free = g*128 + k*64 + j*16 + u*8 + b
  out_dev [sb,128,1024]: part = 32j+p; free = g*32 + k*16 + u*8 + b
"""

import numpy as np

B, C, P, H, W = 8, 32, 32, 128, 128
NCORES = 8
HPER = H // NCORES          # 16
PX = HPER * W               # 2048
NSB = 8                     # super-batches per core
NG = 32                     # groups per super-batch

_DT = np.float32


def _pixelize(arr_t, n):
    """arr_t: [h, w, ...]; returns [sb, g, k, j, ...] for core n's h-strip."""
    a = arr_t[HPER * n:HPER * (n + 1)]
    a = a.reshape(PX, *a.shape[2:])
    return a.reshape(NSB, NG, 2, 4, *a.shape[1:])


def _prepare_inputs(z, filt, bias):
    zr = np.ascontiguousarray(z.real).astype(_DT)
    zi = np.ascontiguousarray(z.imag).astype(_DT)
    fr = np.ascontiguousarray(filt.real).astype(_DT)
    fi = np.ascontiguousarray(filt.imag).astype(_DT)
    br = np.ascontiguousarray(bias.real).astype(_DT)
    bi = np.ascontiguousarray(bias.imag).astype(_DT)

    fr_t = fr.transpose(2, 3, 0, 1)   # [h, w, c, p]
    fi_t = fi.transpose(2, 3, 0, 1)
    br_t = br.transpose(2, 3, 0, 1)
    bi_t = bi.transpose(2, 3, 0, 1)
    zr_t = zr.transpose(2, 3, 1, 0)   # [h, w, c, b]
    zi_t = zi.transpose(2, 3, 1, 0)

    in_maps = []
    for n in range(NCORES):
        # [sb, g, k, j, c, x] -> [sb, c, g, k, j, x]
        perm = (0, 4, 1, 2, 3, 5)
        frp = _pixelize(fr_t, n).transpose(perm)
        fip = _pixelize(fi_t, n).transpose(perm)
        brp = _pixelize(br_t, n).transpose(perm)
        bip = _pixelize(bi_t, n).transpose(perm)
        # rows = (blk4, c32): [fr; br; fi; bi]
        wc = np.stack([frp, brp, fip, bip], axis=1)    # [sb, blk, c, g, k, j, p]
        wc = wc.reshape(NSB, 128, 8192)

        zrp = _pixelize(zr_t, n).transpose(perm)       # [sb, c, g, k, j, b]
        zip_ = _pixelize(zi_t, n).transpose(perm)
        zd = np.empty((NSB, 2, 32, NG, 2, 4, 2, 8), _DT)
        zd[:, 0, ..., 0, :] = zrp
        zd[:, 0, ..., 1, :] = zip_
        zd[:, 1, ..., 0, :] = -zip_
        zd[:, 1, ..., 1, :] = zrp
        zd = zd.reshape(NSB, 2, 32, 4096)

        in_maps.append({
            "wc": np.ascontiguousarray(wc),
            "zd": np.ascontiguousarray(zd),
        })
    return in_maps


def _assemble_output(res_list):
    out = np.empty((B, P, H, W), np.complex64)
    for n in range(NCORES):
        # part q = 32j + p ; free f = g*32 + k*16 + u*8 + b
        arr = res_list[n].reshape(NSB, 4, 32, NG, 2, 2, 8)
        # [sb, j, p, g, k, u, b] -> [b, p, sb, g, k, j, u]
        arr = arr.transpose(6, 2, 0, 3, 4, 1, 5)
        arr = np.ascontiguousarray(arr).reshape(B, P, HPER, W, 2)
        out[:, :, HPER * n:HPER * (n + 1), :] = arr[..., 0] + 1j * arr[..., 1]
    return out


_NC_CACHE = [None]


def _build_bass():
    if _NC_CACHE[0] is not None:
        return _NC_CACHE[0]
    import concourse.mybir as mybir
    import concourse.tile as tile
    from concourse import bacc

    f32 = mybir.dt.float32
    nc = bacc.Bacc("TRN2", target_bir_lowering=False, debug=False)
    wc_d = nc.dram_tensor("wc", [NSB, 128, 8192], f32, kind="ExternalInput")
    zd_d = nc.dram_tensor("zd", [NSB, 2, 32, 4096], f32, kind="ExternalInput")
    out_d = nc.dram_tensor("out_dev", [NSB, 128, 1024], f32,
                           kind="ExternalOutput")

    # Persistent double-buffered z tensors (fixed SBUF allocations, outside
    # the Tile pools so their slots can't be recycled).
    zb_handles = [
        nc.alloc_sbuf_tensor(f"zb{i}", [128, 4096], f32) for i in range(3)
    ]

    with tile.TileContext(nc) as tc:
        with (
            tc.tile_pool(name="wp", bufs=3) as wpool,
            tc.tile_pool(name="op", bufs=3) as opool,
            tc.tile_pool(name="pp", bufs=8, space="PSUM") as pspool,
        ):
            # const rhs rows, written once per z buffer:
            # rows 32-63 : per 16-block [1]*8 + [0]*8   (bias.re -> re col)
            # rows 96-127: per 16-block [0]*8 + [1]*8   (bias.im -> im col)
            zbufs = []
            for i in range(3):
                zb = zb_handles[i][:]
                for rbase, off in ((32, 0), (96, 8)):
                    cr = zb[rbase:rbase + 32]                  # [32, 4096]
                    crq = cr.rearrange("r (m q) -> r m q", q=16)
                    nc.vector.memset(cr, 0.0)
                    nc.vector.memset(crq[:, :, off:off + 8], 1.0)
                zbufs.append(zb)

            for sb in range(NSB):
                wc_t = wpool.tile([128, 8192], f32, name="wc_t", tag="wc_t")
                nc.sync.dma_start(out=wc_t, in_=wc_d[sb])
                zb = zbufs[sb % 3]
                for hf in range(2):
                    nc.sync.dma_start(out=zb[64 * hf:64 * hf + 32, :],
                                      in_=zd_d[sb, hf])

                o_t = opool.tile([128, 1024], f32, name="o_t", tag="o_t")
                for gg in range(2):                 # 16 groups per psum tile
                    # Full-bank psum tile (2048B/partition) so accumulation
                    # zero-regions align exactly with 32-partition col-group
                    # slices.
                    ps = pspool.tile([128, 512], f32, name="ps", tag="ps")
                    for g16 in range(16):
                        g = gg * 16 + g16
                        # one K=128 matmul per pixel; consecutive matmuls hit
                        # different col-groups and run concurrently
                        for k in range(2):
                            for j in range(4):
                                fo = g * 256 + k * 128 + j * 32
                                zo = g * 128 + k * 64 + j * 16
                                lhs = wc_t[:, fo:fo + 32]
                                rhs = zb[:, zo:zo + 16]
                                po = ps[32 * j:32 * j + 32,
                                        g16 * 32 + k * 16:
                                        g16 * 32 + k * 16 + 16]
                                nc.tensor.matmul(
                                    po, lhs, rhs, start=True, stop=True,
                                    tile_position=(0, 32 * j),
                                    # sim's global group-check shadow
                                    # mis-addresses partition-sliced psum
                                    # outputs; the per-tensor pending-zero
                                    # numerics are still modeled faithfully
                                    skip_group_check=True,
                                )
                    nc.vector.tensor_copy(
                        o_t[:, gg * 512:(gg + 1) * 512], ps)
                # output DMA on the ACT HWDGE ring: its compute-completion
                # wait must not head-of-line-block the input DMAs queued on
                # the SP ring
                nc.scalar.dma_start(out=out_d[sb], in_=o_t)

    nc.compile()
    _NC_CACHE[0] = nc
    return nc


def run(z, filt, bias, trace=False, trace_kwargs=None):
    """Returns (out, BassKernelResults)."""
    from concourse.bass_utils import run_bass_kernel_spmd
    in_maps = _prepare_inputs(z, filt, bias)
    nc = _build_bass()
    bkr = run_bass_kernel_spmd(
        nc, in_maps, core_ids=list(range(NCORES)),
        trace=trace, **(trace_kwargs or {}),
    )
    out = _assemble_output([r["out_dev"] for r in bkr.results])
    return out, bkr


def kernel(z, filt, bias):
    out, _ = run(np.asarray(z), np.asarray(filt), np.asarray(bias))
    return out


# revision 16
# speedup vs baseline: 1.7837x; 1.0331x over previous
"""Trainium2 Bass kernel for nn_FFTConv:
  out[b,p,h,w] = sum_c z[b,c,h,w]*filt[c,p,h,w] + sum_c bias[c,p,h,w]
with complex64 z[8,32,128,128], filt/bias[32,32,128,128].

Strategy
--------
Shard the spatial H dim across the 8 NeuronCores (16 rows each) -- pure data
parallelism, zero replication, no collectives.

Each output pixel needs a tiny complex matmul out[p,b] = F(px)^T @ z(px) with
K=c=32, M=p=32, N=b=8, plus a bias-channel sum. We fold the whole complex
product AND the bias reduction into ONE K=128 real matmul per pixel:

  lhsT (128 x 32p) rows = [filt.re(32c); bias.re(32c); filt.im(32c); bias.im(32c)]
  rhs  (128 x 16)  cols = (u2, b8):
      rows  0-31  : [ z.re | z.im ]
      rows 32-63  : [  1   |  0   ]   (bias.re only contributes to re)
      rows 64-95  : [-z.im | z.re ]
      rows 96-127 : [  0   |  1   ]
  out[p, (u,b)]  ->  re = fr*zr - fi*zi + sum(br) ; im = fr*zi + fi*zr + sum(bi)

Pixels are packed 4-at-a-time across the PE's four 32-col sub-array groups
(tile_position=(0, 32j)); the constant ones/zeros rows live in persistent
SBUF buffers written once (memset), so they cost no DMA.

NOTE: concurrent fp32 matmuls from *different* row-groups into the same
col-group crash the PE (fp32 LO/HI two-pass) -- with K=128 every matmul uses
all rows as a single instruction, which is safe; col-groups are concurrent.

Engine split: input DMAs on the SP HWDGE ring, output DMAs on the ACT ring
(whose compute-completion waits must not head-of-line-block input prefetch),
psum evacuation on DVE.

Host-side numpy only reorders data (transpose/interleave/negate) into
pixel-major DMA-friendly layouts; all arithmetic happens on device.

Layouts (per core, f32):
  local pixel px = h_local*128 + w in [0,2048); group g = px//8;
  within group: j = px%4 (col slot), k = (px%8)//4 (serial);
  super-batch sb = g//16 (16 sbs x 16 groups x 8 px).
  wc [sb,128,4096]: row = 32*blk + c (blk: fr,br,fi,bi); free = g*256+k*128+j*32+p
  zd [sb,2,32,2048]: half 0 -> SBUF rows 0-31 (u0=zr,u1=zi),
                     half 1 -> rows 64-95 (u0=-zi,u1=zr);
                     free = g*128 + k*64 + j*16 + u*8 + b
  out_dev [sb,128,512]: part = 32j+p; free = g*32 + k*16 + u*8 + b
"""

import numpy as np

B, C, P, H, W = 8, 32, 32, 128, 128
NCORES = 8
HPER = H // NCORES          # 16
PX = HPER * W               # 2048
NSB = 16                    # super-batches per core
NG = 16                     # groups per super-batch
WFREE = NG * 256            # 4096
ZFREE = NG * 128            # 2048
OFREE = NG * 32             # 512

_DT = np.float32


def _pixelize(arr_t, n):
    """arr_t: [h, w, ...]; returns [sb, g, k, j, ...] for core n's h-strip."""
    a = arr_t[HPER * n:HPER * (n + 1)]
    a = a.reshape(PX, *a.shape[2:])
    return a.reshape(NSB, NG, 2, 4, *a.shape[1:])


def _prepare_inputs(z, filt, bias):
    zr = np.ascontiguousarray(z.real).astype(_DT)
    zi = np.ascontiguousarray(z.imag).astype(_DT)
    fr = np.ascontiguousarray(filt.real).astype(_DT)
    fi = np.ascontiguousarray(filt.imag).astype(_DT)
    br = np.ascontiguousarray(bias.real).astype(_DT)
    bi = np.ascontiguousarray(bias.imag).astype(_DT)

    fr_t = fr.transpose(2, 3, 0, 1)   # [h, w, c, p]
    fi_t = fi.transpose(2, 3, 0, 1)
    br_t = br.transpose(2, 3, 0, 1)
    bi_t = bi.transpose(2, 3, 0, 1)
    zr_t = zr.transpose(2, 3, 1, 0)   # [h, w, c, b]
    zi_t = zi.transpose(2, 3, 1, 0)

    in_maps = []
    for n in range(NCORES):
        # [sb, g, k, j, c, x] -> [sb, c, g, k, j, x]
        perm = (0, 4, 1, 2, 3, 5)
        frp = _pixelize(fr_t, n).transpose(perm)
        fip = _pixelize(fi_t, n).transpose(perm)
        brp = _pixelize(br_t, n).transpose(perm)
        bip = _pixelize(bi_t, n).transpose(perm)
        # rows = (blk4, c32): [fr; br; fi; bi]
        wc = np.stack([frp, brp, fip, bip], axis=1)    # [sb, blk, c, g, k, j, p]
        wc = wc.reshape(NSB, 128, WFREE)

        zrp = _pixelize(zr_t, n).transpose(perm)       # [sb, c, g, k, j, b]
        zip_ = _pixelize(zi_t, n).transpose(perm)
        zd = np.empty((NSB, 2, 32, NG, 2, 4, 2, 8), _DT)
        zd[:, 0, ..., 0, :] = zrp
        zd[:, 0, ..., 1, :] = zip_
        zd[:, 1, ..., 0, :] = -zip_
        zd[:, 1, ..., 1, :] = zrp
        zd = zd.reshape(NSB, 2, 32, ZFREE)

        in_maps.append({
            "wc": np.ascontiguousarray(wc),
            "zd": np.ascontiguousarray(zd),
        })
    return in_maps


def _assemble_output(res_list):
    out = np.empty((B, P, H, W), np.complex64)
    for n in range(NCORES):
        # part q = 32j + p ; free f = g*32 + k*16 + u*8 + b
        arr = res_list[n].reshape(NSB, 4, 32, NG, 2, 2, 8)
        # [sb, j, p, g, k, u, b] -> [b, p, sb, g, k, j, u]
        arr = arr.transpose(6, 2, 0, 3, 4, 1, 5)
        arr = np.ascontiguousarray(arr).reshape(B, P, HPER, W, 2)
        out[:, :, HPER * n:HPER * (n + 1), :] = arr[..., 0] + 1j * arr[..., 1]
    return out


_NC_CACHE = [None]


def _build_bass():
    if _NC_CACHE[0] is not None:
        return _NC_CACHE[0]
    import concourse.mybir as mybir
    import concourse.tile as tile
    from concourse import bacc

    f32 = mybir.dt.float32
    nc = bacc.Bacc("TRN2", target_bir_lowering=False, debug=False)
    wc_d = nc.dram_tensor("wc", [NSB, 128, WFREE], f32, kind="ExternalInput")
    zd_d = nc.dram_tensor("zd", [NSB, 2, 32, ZFREE], f32, kind="ExternalInput")
    out_d = nc.dram_tensor("out_dev", [NSB, 128, OFREE], f32,
                           kind="ExternalOutput")

    # Persistent triple-buffered z tensors (fixed SBUF allocations, outside
    # the Tile pools so their slots can't be recycled).
    zb_handles = [
        nc.alloc_sbuf_tensor(f"zb{i}", [128, ZFREE], f32) for i in range(3)
    ]

    with tile.TileContext(nc) as tc:
        with (
            tc.tile_pool(name="wp", bufs=4) as wpool,
            tc.tile_pool(name="op", bufs=4) as opool,
            tc.tile_pool(name="pp", bufs=8, space="PSUM") as pspool,
        ):
            # const rhs rows, written once per z buffer:
            # rows 32-63 : per 16-block [1]*8 + [0]*8   (bias.re -> re col)
            # rows 96-127: per 16-block [0]*8 + [1]*8   (bias.im -> im col)
            zbufs = []
            for i in range(3):
                zb = zb_handles[i][:]
                for rbase, off in ((32, 0), (96, 8)):
                    cr = zb[rbase:rbase + 32]                  # [32, ZFREE]
                    crq = cr.rearrange("r (m q) -> r m q", q=16)
                    nc.vector.memset(cr, 0.0)
                    nc.vector.memset(crq[:, :, off:off + 8], 1.0)
                zbufs.append(zb)

            for sb in range(NSB):
                wc_t = wpool.tile([128, WFREE], f32, name="wc_t", tag="wc_t")
                nc.sync.dma_start(out=wc_t, in_=wc_d[sb])
                zb = zbufs[sb % 3]
                for hf in range(2):
                    nc.sync.dma_start(out=zb[64 * hf:64 * hf + 32, :],
                                      in_=zd_d[sb, hf])

                o_t = opool.tile([128, OFREE], f32, name="o_t", tag="o_t")
                # Full-bank psum tile (2048B/partition) so accumulation
                # zero-regions align exactly with 32-partition col-group
                # slices.
                ps = pspool.tile([128, 512], f32, name="ps", tag="ps")
                for g in range(NG):
                    # one K=128 matmul per pixel; consecutive matmuls hit
                    # different col-groups and run concurrently
                    for k in range(2):
                        for j in range(4):
                            fo = g * 256 + k * 128 + j * 32
                            zo = g * 128 + k * 64 + j * 16
                            lhs = wc_t[:, fo:fo + 32]
                            rhs = zb[:, zo:zo + 16]
                            po = ps[32 * j:32 * j + 32,
                                    g * 32 + k * 16:g * 32 + k * 16 + 16]
                            nc.tensor.matmul(
                                po, lhs, rhs, start=True, stop=True,
                                tile_position=(0, 32 * j),
                                # sim's global group-check shadow
                                # mis-addresses partition-sliced psum
                                # outputs; the per-tensor pending-zero
                                # numerics are still modeled faithfully
                                skip_group_check=True,
                            )
                nc.vector.tensor_copy(o_t, ps)
                # output DMA on the ACT HWDGE ring: its compute-completion
                # wait must not head-of-line-block the input DMAs queued on
                # the SP ring
                nc.scalar.dma_start(out=out_d[sb], in_=o_t)

    nc.compile()
    _NC_CACHE[0] = nc
    return nc


def run(z, filt, bias, trace=False, trace_kwargs=None):
    """Returns (out, BassKernelResults)."""
    from concourse.bass_utils import run_bass_kernel_spmd
    in_maps = _prepare_inputs(z, filt, bias)
    nc = _build_bass()
    bkr = run_bass_kernel_spmd(
        nc, in_maps, core_ids=list(range(NCORES)),
        trace=trace, **(trace_kwargs or {}),
    )
    out = _assemble_output([r["out_dev"] for r in bkr.results])
    return out, bkr


def kernel(z, filt, bias):
    out, _ = run(np.asarray(z), np.asarray(filt), np.asarray(bias))
    return out
